# revision 45
# baseline (speedup 1.0000x reference)
"""Causal attention layer (B=4, N=2048, C=1024, H=16, D=64) on 8 TRN2 NeuronCores.

Sharding: core c -> (batch b = c//2, head-group g = c%2 of 8 heads).
All matmul operands are bf16 (fp32 lowers to two-pass fp32_mode=HIGH matmuls at
~1.5x the cost and defeats fast-weight-load).

Per core, for its (b, g), heads are processed as 4 pairs p (heads 2p, 2p+1):
  qT/kT[pair]  = wT_pair.T-contract(x)          [128 chan, N tok] bf16
  v            = x.T-contract(wv)               token-major [128 tok, 512 chan]
                 copied straight into ve_all[key, nt, head, 64:128] (no PE
                 transposes; col 0 of each 128-block holds ones for denominators)
  S2[k-tile]   = [kT[h0].T@qT[h0] | kT[h1].T@qT[h1]]   two K=64 matmuls on
                 disjoint PE row groups (tile_position (0,0)/(64,0)) -> overlap
  P2           = exp(S2 * D^-0.5) bf16, causal-masked on diagonal tiles
  oT[:,512h]  += ve_all[:,k,2p+h,:].T @ P2[:,512h]   row0 = denom, rows 64:128 = out
  attn_outT    = oT[64:128] * bcast(1/oT[0])    (recip + gpsimd partition_broadcast)
  out_part     = attn_outT.T-contract(projT)
Host sums the two head-group partials per batch and adds proj_b.

QKV chains for pair p+1, v chains, and the output projection are queued as
fillers and paced into the attention stream so the PE stays dense while ScalarE
runs exp.
"""
import sys

sys.path.insert(0, "/opt/trn_rl_repo")

import numpy as np

import concourse.bass as bass  # noqa: F401
import concourse.tile as tile
from concourse import bacc, mybir
from concourse.bass_utils import run_bass_kernel_spmd

F32 = mybir.dt.float32
BF16 = mybir.dt.bfloat16
EXP = mybir.ActivationFunctionType.Exp

B, N, C, H, D = 4, 2048, 1024, 16, 64
G = 8            # heads per core
GC = G * D       # 512 channels per core
NT = N // 128    # 16 row tiles
NS = N // 512    # 4 row supers
CK = C // 128    # 8 contraction chunks

_cache = {}


def _build_nc():
    from contextlib import ExitStack

    nc = bacc.Bacc("TRN2", target_bir_lowering=False, debug=False)

    xT_d = nc.dram_tensor("xT", [C, N], BF16, kind="ExternalInput")
    wqkvT_d = nc.dram_tensor("wqkvT", [C, 3 * GC], BF16, kind="ExternalInput")
    projT_d = nc.dram_tensor("projT", [GC, C], BF16, kind="ExternalInput")
    tril_d = nc.dram_tensor("tril", [128, 128], BF16, kind="ExternalInput")
    out_d = nc.dram_tensor("out", [N, C], F32, kind="ExternalOutput")

    with tile.TileContext(nc) as tc:
        with ExitStack() as ctx:
            consts = ctx.enter_context(tc.tile_pool(name="consts", bufs=1))
            qk_pool = ctx.enter_context(tc.tile_pool(name="qk", bufs=4))
            ve_pool = ctx.enter_context(tc.tile_pool(name="ve", bufs=1))
            wv_pool = ctx.enter_context(tc.tile_pool(name="wv", bufs=1))
            w_pool = ctx.enter_context(tc.tile_pool(name="wA", bufs=4))
            xT_pool = ctx.enter_context(tc.tile_pool(name="xT", bufs=1))
            rf_pool = ctx.enter_context(tc.tile_pool(name="rf", bufs=2))
            bcs_pool = ctx.enter_context(tc.tile_pool(name="bcs", bufs=2))
            tmp_pool = ctx.enter_context(tc.tile_pool(name="tmp", bufs=2))
            oTs_pool = ctx.enter_context(tc.tile_pool(name="oTs", bufs=2))
            ob_pool = ctx.enter_context(tc.tile_pool(name="ob", bufs=2))
            pj_pool = ctx.enter_context(tc.tile_pool(name="pj", bufs=1))
            aoT_pool = ctx.enter_context(tc.tile_pool(name="aoT", bufs=1))
            pt_pool = ctx.enter_context(tc.tile_pool(name="pt", bufs=5))
            psq = ctx.enter_context(tc.tile_pool(name="psq", bufs=2, space="PSUM"))
            psS = ctx.enter_context(tc.tile_pool(name="psS", bufs=2, space="PSUM"))
            psO = ctx.enter_context(tc.tile_pool(name="psO", bufs=1, space="PSUM"))

            tril_sb = consts.tile([128, 128], BF16)
            nc.sync.dma_start(tril_sb[:], tril_d[:])

            wv_sb = wv_pool.tile([128, CK, GC], BF16, name="wv")

            def load_w(ot):
                wt = w_pool.tile([128, CK, 128], BF16, tag="wt", name=f"wt{ot}")
                src = wqkvT_d[:, 128 * ot:128 * (ot + 1)].rearrange(
                    "(cc p) o -> p cc o", p=128
                )
                nc.sync.dma_start(wt[:], src)
                return wt

            def x_sup_dma(sup):
                nc.sync.dma_start(
                    xs_all[:, :, 512 * sup:512 * (sup + 1)],
                    xT_d[:, 512 * sup:512 * (sup + 1)].rearrange(
                        "(cc p) n -> p cc n", p=128
                    ),
                )

            # one big DMA per x super (fans out across the 16 HW queues);
            # pair-0 weights first so the first q/k chain can start ASAP
            xs_all = xT_pool.tile([128, CK, N], BF16, name="xs_all")
            wq0 = load_w(0)
            for qtr in range(4):
                nc.sync.dma_start(
                    xs_all[:, 2 * qtr:2 * (qtr + 1), 0:512],
                    xT_d[256 * qtr:256 * (qtr + 1), 0:512].rearrange(
                        "(cc p) n -> p cc n", p=128
                    ),
                )
            wk0 = load_w(4)
            for half in (0, 1):
                nc.sync.dma_start(
                    wv_sb[:, 4 * half:4 * (half + 1), :],
                    wqkvT_d[512 * half:512 * (half + 1), 2 * GC:3 * GC].rearrange(
                        "(cc p) o -> p cc o", p=128
                    ),
                )
            for sup in range(1, NS):
                x_sup_dma(sup)

            pj_all = pj_pool.tile([128, 4, C], BF16, name="pj_all")
            nc.sync.dma_start(
                pj_all[:], projT_d[:].rearrange("(ac p) o -> p ac o", p=128)
            )

            # ve_all[key, nt, head, col]: col 0 = ones, cols 64:128 = v
            ve_all = ve_pool.tile([128, NT, G, 128], BF16, name="ve")
            nc.vector.memset(ve_all[:, :, :, 0:1], 1.0)

            def v_chain(nt):
                psv = psq.tile([128, GC], F32, tag="qa", name=f"psv{nt}")
                for cc in range(CK):
                    nc.tensor.matmul(
                        psv[:],
                        xs_all[:, cc, 128 * nt:128 * (nt + 1)],
                        wv_sb[:, cc, :],
                        start=(cc == 0),
                        stop=(cc == CK - 1),
                    )
                nc.vector.tensor_copy(
                    ve_all[:, nt, :, 64:128],
                    psv[:].rearrange("p (h d) -> p h d", h=G),
                )

            def qk_chain(wt, dst, sup):
                pq = psq.tile([128, 512], F32, tag="qa", name="pq")
                for cc in range(CK):
                    nc.tensor.matmul(
                        pq[:],
                        wt[:, cc, :],
                        xs_all[:, cc, 512 * sup:512 * (sup + 1)],
                        start=(cc == 0),
                        stop=(cc == CK - 1),
                    )
                nc.vector.tensor_copy(dst[:, 512 * sup:512 * (sup + 1)], pq[:])

            attn_outT = [aoT_pool.tile([128, N], BF16, tag=f"ao{p}", name=f"ao{p}")
                         for p in range(4)]

            def proj_step(nt):
                for oc in (0, 1):
                    pp = psq.tile([128, 512], F32, tag="qa", name="pp")
                    for ac in range(4):
                        nc.tensor.matmul(
                            pp[:],
                            attn_outT[ac][:, 128 * nt:128 * (nt + 1)],
                            pj_all[:, ac, 512 * oc:512 * (oc + 1)],
                            start=(ac == 0),
                            stop=(ac == 3),
                        )
                    ob = ob_pool.tile([128, 512], F32, tag="ob", name="ob")
                    nc.vector.tensor_copy(ob[:], pp[:])
                    nc.sync.dma_start(
                        out_d[128 * nt:128 * (nt + 1), 512 * oc:512 * (oc + 1)],
                        ob[:],
                    )

            # super-3 proj is split so only the pair-3 contribution remains
            # after the last attention super (short tail)
            pproj_sb = consts.tile([128, 4, 2, 512], F32, tag="pproj",
                                   name="pproj")

            def proj_partial(nt):
                for oc in (0, 1):
                    pp = psq.tile([128, 512], F32, tag="qa", name="pp")
                    for ac in range(3):
                        nc.tensor.matmul(
                            pp[:],
                            attn_outT[ac][:, 128 * nt:128 * (nt + 1)],
                            pj_all[:, ac, 512 * oc:512 * (oc + 1)],
                            start=(ac == 0),
                            stop=(ac == 2),
                        )
                    nc.vector.tensor_copy(pproj_sb[:, nt - 12, oc, :], pp[:])

            def proj_tail_step(nt, pool, tag):
                # full proj for one token tile with zero Vector involvement:
                # both oc chains into one 1024-wide psum, evacuated on the
                # (tail-idle) Scalar engine
                pp = pool.tile([128, 1024], F32, tag=tag, name="ppt")
                for oc in (0, 1):
                    for ac in range(4):
                        nc.tensor.matmul(
                            pp[:, 512 * oc:512 * (oc + 1)],
                            attn_outT[ac][:, 128 * nt:128 * (nt + 1)],
                            pj_all[:, ac, 512 * oc:512 * (oc + 1)],
                            start=(ac == 0),
                            stop=(ac == 3),
                        )
                ob = ob_pool.tile([128, 1024], F32, tag="ob2", name="obt")
                nc.scalar.copy(ob[:], pp[:])
                nc.sync.dma_start(out_d[128 * nt:128 * (nt + 1), :], ob[:])

            def proj_final(nt, pool, tag):
                pp = pool.tile([128, 1024], F32, tag=tag, name="pp")
                for oc in (0, 1):
                    nc.tensor.matmul(
                        pp[:, 512 * oc:512 * (oc + 1)],
                        attn_outT[3][:, 128 * nt:128 * (nt + 1)],
                        pj_all[:, 3, 512 * oc:512 * (oc + 1)],
                    )
                ob = ob_pool.tile([128, 1024], F32, tag="ob2", name="ob2")
                nc.vector.tensor_add(
                    ob[:], pp[:],
                    pproj_sb[:, nt - 12, :, :].rearrange("p a q -> p (a q)"),
                )
                nc.sync.dma_start(out_d[128 * nt:128 * (nt + 1), :], ob[:])

            # ---------------- filler machinery ----------------
            pending = []
            state = {}

            def fill(n):
                done = 0
                while pending and done < n:
                    kind, fn = pending.pop(0)
                    if kind == "w":
                        fn()
                    else:
                        fn()
                        done += 1

            def fill_all():
                while pending:
                    fill(4)

            def queue_pair(p):
                """Queue q/k chains for pair p (weights + 8 chains)."""
                qT = qk_pool.tile([128, N], BF16, tag="qk", name=f"q{p}")
                kT = qk_pool.tile([128, N], BF16, tag="qk", name=f"k{p}")

                def _wq():
                    state[f"wq{p}"] = load_w(p)

                def _wk():
                    state[f"wk{p}"] = load_w(4 + p)

                steps = [("w", _wq), ("w", _wk)]
                for sup in range(NS):
                    for key, dst in ((f"wq{p}", qT), (f"wk{p}", kT)):
                        def _c(key=key, dst=dst, sup=sup):
                            qk_chain(state[key], dst, sup)
                        steps.append(("q", _c))
                return qT, kT, steps

            # ---------------- prologue (q/k first: less DMA to wait on) ----
            qT = qk_pool.tile([128, N], BF16, tag="qk", name="q0")
            kT = qk_pool.tile([128, N], BF16, tag="qk", name="k0")
            qk_chain(wq0, qT, 0)
            qk_chain(wk0, kT, 0)
            for nt in range(4):
                v_chain(nt)
            for sup in range(1, NS):
                for wt, dst in ((wq0, qT), (wk0, kT)):
                    def _c(wt=wt, dst=dst, sup=sup):
                        qk_chain(wt, dst, sup)
                    pending.append(("q", _c))

            # ---------------- attention pair loop ----------------
            tail_fill = []
            deferred = []
            for p in range(4):
                if p < 3:
                    nq, nk, nsteps = queue_pair(p + 1)
                    pending.extend(nsteps)

                for s in range(NS):
                    if p == 0 and s > 0:
                        for nt in range(4 * s, 4 * s + 4):
                            v_chain(nt)
                    nkb = 4 * (s + 1)
                    oT = psO.tile([128, 1024], F32, tag="oT", name="oT")
                    P_prev = None
                    def pv_pair(kp, P):
                        # diagonal tiles only touch queries >= 128*r (causal)
                        q0 = 128 * max(kp - 4 * s, 0)
                        for h in (0, 1):
                            c0 = 512 * h
                            nc.tensor.matmul(
                                oT[:, c0 + q0:c0 + 512],
                                ve_all[:, kp, 2 * p + h, :],
                                P[:, c0 + q0:c0 + 512],
                                start=(kp == 0),
                                stop=(kp == nkb - 1),
                            )

                    ran_mid = False
                    for k in range(nkb):
                        if k == 2 and deferred:
                            deferred[0][0]()
                            ran_mid = True
                        if k == 5 and ran_mid and deferred:
                            deferred.pop(0)[1]()
                        if pending and (s == 0 or
                                        k % (4 if p < 3 else 2) == 0):
                            fill(1)
                        r = k - 4 * s
                        q0 = 128 * max(r, 0)
                        S2 = psS.tile([128, 1024], F32, tag="s2", name="S2")
                        for h in (0, 1):
                            hh = slice(64 * h, 64 * (h + 1))
                            nc.tensor.matmul(
                                S2[:, 512 * h + q0:512 * (h + 1)],
                                kT[hh, 128 * k:128 * (k + 1)],
                                qT[hh, 512 * s + q0:512 * (s + 1)],
                            )
                        P2 = pt_pool.tile([128, 1024], BF16, tag="pt", name="P2")
                        s3 = S2[:].rearrange("p (h q) -> p h q", h=2)
                        p3 = P2[:].rearrange("p (h q) -> p h q", h=2)
                        nc.scalar.activation(
                            p3[:, :, q0:512], s3[:, :, q0:512], EXP,
                            scale=float(D) ** -0.5,
                        )
                        if r >= 0:
                            for h in (0, 1):
                                c0 = 512 * h
                                nc.vector.tensor_mul(
                                    P2[:, c0 + q0:c0 + q0 + 128],
                                    P2[:, c0 + q0:c0 + q0 + 128],
                                    tril_sb[:],
                                )
                        if P_prev is not None:
                            pv_pair(k - 1, P_prev)
                        P_prev = P2
                    pv_pair(nkb - 1, P_prev)
                    if ran_mid and deferred:  # short super: stage2 after loop
                        deferred.pop(0)[1]()
                    last = p == 3 and s == NS - 1
                    if not last:
                        # evacuate psum fast; recip+broadcast and mul+DMA are
                        # deferred into the next super (two stages) so the
                        # Vector FIFO never head-of-line blocks on them
                        oTs = oTs_pool.tile([128, 1024], F32, tag="oTs",
                                            name="oTs")
                        nc.vector.tensor_copy(oTs[:], oT[:])
                        bcs = bcs_pool.tile([128, 1024], F32, tag="bcs",
                                            name="bcs")

                        def _norm_mid(oTs=oTs, bcs=bcs):
                            Rf = rf_pool.tile([1, 1024], F32, tag="rf",
                                              name="Rf")
                            nc.vector.reciprocal_approx_fast(Rf[:], oTs[0:1, :])
                            nc.gpsimd.partition_broadcast(bcs[:], Rf[:])

                        def _norm_tail(p=p, s=s, oTs=oTs, bcs=bcs):
                            tmp = tmp_pool.tile([128, 1024], BF16, tag="tmp",
                                                name="tmp")
                            nc.vector.tensor_mul(
                                tmp[64:128, :], oTs[64:128, :], bcs[64:128, :]
                            )
                            for h in (0, 1):
                                nc.sync.dma_start(
                                    attn_outT[p][64 * h:64 * (h + 1),
                                                 512 * s:512 * (s + 1)],
                                    tmp[64:128, 512 * h:512 * (h + 1)],
                                )
                            if p == 2 and s == NS - 1:
                                # ao[0..2] complete: super-3 partial proj
                                for nt in range(12, 16):
                                    def _pp(nt=nt):
                                        proj_partial(nt)
                                    pending.append(("p", _pp))
                            if p == 3 and s < NS - 1:
                                for nt in range(4 * s, 4 * s + 4):
                                    if s == 2 and nt >= 9:
                                        def _pt(nt=nt):
                                            proj_tail_step(nt, psS, "s2")
                                        tail_fill.append(_pt)
                                    else:
                                        def _pj(nt=nt):
                                            proj_step(nt)
                                        pending.append(("p", _pj))
                        deferred.append((_norm_mid, _norm_tail))
                    else:
                        # last super: psum-direct normalize in 128-token
                        # chunks; reserved proj steps (ScalarE evacuation, no
                        # Vector deps) keep the PE busy through the chain
                        if tail_fill:
                            tail_fill.pop(0)()
                        Rf = rf_pool.tile([1, 1024], F32, tag="rf", name="Rf")
                        nc.vector.reciprocal_approx_fast(Rf[:], oT[0:1, :])
                        bcs = bcs_pool.tile([128, 1024], F32, tag="bcs",
                                            name="bcs")
                        oT3 = oT[:].rearrange("p (h q) -> p h q", h=2)
                        Rf3 = Rf[:].rearrange("o (h q) -> o h q", h=2)
                        bc3 = bcs[:].rearrange("p (h q) -> p h q", h=2)
                        tmp = tmp_pool.tile([128, 1024], BF16, tag="tmp",
                                            name="tmp")
                        t3 = tmp[:].rearrange("p (h q) -> p h q", h=2)
                        for i, nt in enumerate(range(12, 16)):
                            cs = slice(128 * i, 128 * (i + 1))
                            nc.gpsimd.partition_broadcast(
                                bc3[:, :, cs], Rf3[:, :, cs]
                            )
                            nc.vector.tensor_mul(
                                t3[64:128, :, cs], oT3[64:128, :, cs],
                                bc3[64:128, :, cs]
                            )
                            for h in (0, 1):
                                nc.sync.dma_start(
                                    attn_outT[3][64 * h:64 * (h + 1),
                                                 1536 + 128 * i:1664 + 128 * i],
                                    tmp[64:128, 512 * h + 128 * i:
                                        512 * h + 128 * (i + 1)],
                                )
                            if i == 0:  # all reserved steps before finals:
                                while tail_fill:  # in-order PE would block
                                    tail_fill.pop(0)()  # them behind F12
                            proj_final(nt, psS, "s2")
                if p < 3:
                    qT, kT = nq, nk
            for fn in tail_fill:
                fn()
            fill_all()

    nc.compile()
    return nc


def _tril_np():
    import ml_dtypes

    i = np.arange(128)[:, None]
    j = np.arange(128)[None, :]
    return (j >= i).astype(np.float32).astype(ml_dtypes.bfloat16)


def make_in_maps(x, qkv_w, proj_w):
    import ml_dtypes

    bf16 = ml_dtypes.bfloat16
    x = np.asarray(x, dtype=np.float32)
    qkv_w = np.asarray(qkv_w, dtype=np.float32)
    proj_w = np.asarray(proj_w, dtype=np.float32)
    tril = _tril_np()
    in_maps = []
    for c in range(8):
        b, g = c // 2, c % 2
        sl = slice(g * GC, (g + 1) * GC)
        wq, wk, wv = qkv_w[0:C][sl], qkv_w[C:2 * C][sl], qkv_w[2 * C:3 * C][sl]
        in_maps.append(
            {
                "xT": np.ascontiguousarray(x[b].T).astype(bf16),
                "wqkvT": np.ascontiguousarray(
                    np.concatenate([wq, wk, wv], 0).T
                ).astype(bf16),
                "projT": np.ascontiguousarray(proj_w[:, sl].T).astype(bf16),
                "tril": tril,
            }
        )
    return in_maps


def kernel(x, qkv_w, proj_w, proj_b):
    proj_b = np.asarray(proj_b, dtype=np.float32)

    if "nc" not in _cache:
        _cache["nc"] = _build_nc()
    nc = _cache["nc"]

    in_maps = make_in_maps(x, qkv_w, proj_w)
    res = run_bass_kernel_spmd(nc, in_maps, core_ids=list(range(8)))
    out = np.stack(
        [res.results[2 * b]["out"] + res.results[2 * b + 1]["out"] for b in range(B)], 0
    )
    return (out + proj_b[None, None, :]).astype(np.float32)


# revision 47
# speedup vs baseline: 1.0139x; 1.0139x over previous
"""Causal attention layer (B=4, N=2048, C=1024, H=16, D=64) on 8 TRN2 NeuronCores.

Sharding: core c -> (batch b = c//2, head-group g = c%2 of 8 heads).
All matmul operands are bf16 (fp32 lowers to two-pass fp32_mode=HIGH matmuls at
~1.5x the cost and defeats fast-weight-load).

Per core, for its (b, g), heads are processed as 4 pairs p (heads 2p, 2p+1):
  qT/kT[pair]  = wT_pair.T-contract(x)          [128 chan, N tok] bf16
  v            = x.T-contract(wv)               token-major [128 tok, 512 chan]
                 copied straight into ve_all[key, nt, head, 64:128] (no PE
                 transposes; col 0 of each 128-block holds ones for denominators)
  S2[k-tile]   = [kT[h0].T@qT[h0] | kT[h1].T@qT[h1]]   two K=64 matmuls on
                 disjoint PE row groups (tile_position (0,0)/(64,0)) -> overlap
  P2           = exp(S2 * D^-0.5) bf16, causal-masked on diagonal tiles
  oT[:,512h]  += ve_all[:,k,2p+h,:].T @ P2[:,512h]   row0 = denom, rows 64:128 = out
  attn_outT    = oT[64:128] * bcast(1/oT[0])    (recip + gpsimd partition_broadcast)
  out_part     = attn_outT.T-contract(projT)
Host sums the two head-group partials per batch and adds proj_b.

QKV chains for pair p+1, v chains, and the output projection are queued as
fillers and paced into the attention stream so the PE stays dense while ScalarE
runs exp.
"""
import sys

sys.path.insert(0, "/opt/trn_rl_repo")

import numpy as np

import concourse.bass as bass  # noqa: F401
import concourse.tile as tile
from concourse import bacc, mybir
from concourse.bass_utils import run_bass_kernel_spmd

F32 = mybir.dt.float32
BF16 = mybir.dt.bfloat16
EXP = mybir.ActivationFunctionType.Exp

B, N, C, H, D = 4, 2048, 1024, 16, 64
G = 8            # heads per core
GC = G * D       # 512 channels per core
NT = N // 128    # 16 row tiles
NS = N // 512    # 4 row supers
CK = C // 128    # 8 contraction chunks

_cache = {}


def _build_nc():
    from contextlib import ExitStack

    nc = bacc.Bacc("TRN2", target_bir_lowering=False, debug=False)

    xT_d = nc.dram_tensor("xT", [C, N], BF16, kind="ExternalInput")
    wqkvT_d = nc.dram_tensor("wqkvT", [C, 3 * GC], BF16, kind="ExternalInput")
    projT_d = nc.dram_tensor("projT", [GC, C], BF16, kind="ExternalInput")
    tril_d = nc.dram_tensor("tril", [128, 128], BF16, kind="ExternalInput")
    out_d = nc.dram_tensor("out", [N, C], F32, kind="ExternalOutput")

    with tile.TileContext(nc) as tc:
        with ExitStack() as ctx:
            consts = ctx.enter_context(tc.tile_pool(name="consts", bufs=1))
            qk_pool = ctx.enter_context(tc.tile_pool(name="qk", bufs=4))
            ve_pool = ctx.enter_context(tc.tile_pool(name="ve", bufs=1))
            wv_pool = ctx.enter_context(tc.tile_pool(name="wv", bufs=1))
            w_pool = ctx.enter_context(tc.tile_pool(name="wA", bufs=4))
            xT_pool = ctx.enter_context(tc.tile_pool(name="xT", bufs=1))
            rf_pool = ctx.enter_context(tc.tile_pool(name="rf", bufs=2))
            bcs_pool = ctx.enter_context(tc.tile_pool(name="bcs", bufs=2))
            tmp_pool = ctx.enter_context(tc.tile_pool(name="tmp", bufs=2))
            oTs_pool = ctx.enter_context(tc.tile_pool(name="oTs", bufs=2))
            ob_pool = ctx.enter_context(tc.tile_pool(name="ob", bufs=2))
            pj_pool = ctx.enter_context(tc.tile_pool(name="pj", bufs=1))
            aoT_pool = ctx.enter_context(tc.tile_pool(name="aoT", bufs=1))
            pt_pool = ctx.enter_context(tc.tile_pool(name="pt", bufs=4))
            psq = ctx.enter_context(tc.tile_pool(name="psq", bufs=2, space="PSUM"))
            psS = ctx.enter_context(tc.tile_pool(name="psS", bufs=2, space="PSUM"))
            psO = ctx.enter_context(tc.tile_pool(name="psO", bufs=1, space="PSUM"))

            tril_sb = consts.tile([128, 128], BF16)
            nc.sync.dma_start(tril_sb[:], tril_d[:])

            wv_sb = wv_pool.tile([128, CK, GC], BF16, name="wv")

            def load_w(ot):
                wt = w_pool.tile([128, CK, 128], BF16, tag="wt", name=f"wt{ot}")
                src = wqkvT_d[:, 128 * ot:128 * (ot + 1)].rearrange(
                    "(cc p) o -> p cc o", p=128
                )
                nc.sync.dma_start(wt[:], src)
                return wt

            def x_sup_dma(sup):
                nc.sync.dma_start(
                    xs_all[:, :, 512 * sup:512 * (sup + 1)],
                    xT_d[:, 512 * sup:512 * (sup + 1)].rearrange(
                        "(cc p) n -> p cc n", p=128
                    ),
                )

            # one big DMA per x super (fans out across the 16 HW queues);
            # pair-0 weights first so the first q/k chain can start ASAP
            xs_all = xT_pool.tile([128, CK, N], BF16, name="xs_all")
            wq0 = load_w(0)
            for qtr in range(4):
                nc.sync.dma_start(
                    xs_all[:, 2 * qtr:2 * (qtr + 1), 0:512],
                    xT_d[256 * qtr:256 * (qtr + 1), 0:512].rearrange(
                        "(cc p) n -> p cc n", p=128
                    ),
                )
            wk0 = load_w(4)
            for half in (0, 1):
                nc.sync.dma_start(
                    wv_sb[:, 4 * half:4 * (half + 1), :],
                    wqkvT_d[512 * half:512 * (half + 1), 2 * GC:3 * GC].rearrange(
                        "(cc p) o -> p cc o", p=128
                    ),
                )
            for sup in range(1, NS):
                x_sup_dma(sup)

            pj_all = pj_pool.tile([128, 4, C], BF16, name="pj_all")
            nc.sync.dma_start(
                pj_all[:], projT_d[:].rearrange("(ac p) o -> p ac o", p=128)
            )

            # ve_all[key, nt, head, col]: col 0 = ones, cols 64:128 = v
            ve_all = ve_pool.tile([128, NT, G, 128], BF16, name="ve")
            nc.vector.memset(ve_all[:, :, :, 0:1], 1.0)

            def v_chain(nt):
                psv = psq.tile([128, GC], F32, tag="qa", name=f"psv{nt}")
                for cc in range(CK):
                    nc.tensor.matmul(
                        psv[:],
                        xs_all[:, cc, 128 * nt:128 * (nt + 1)],
                        wv_sb[:, cc, :],
                        start=(cc == 0),
                        stop=(cc == CK - 1),
                    )
                nc.vector.tensor_copy(
                    ve_all[:, nt, :, 64:128],
                    psv[:].rearrange("p (h d) -> p h d", h=G),
                )

            def qk_chain(wt, dst, sup):
                pq = psq.tile([128, 512], F32, tag="qa", name="pq")
                for cc in range(CK):
                    nc.tensor.matmul(
                        pq[:],
                        wt[:, cc, :],
                        xs_all[:, cc, 512 * sup:512 * (sup + 1)],
                        start=(cc == 0),
                        stop=(cc == CK - 1),
                    )
                nc.vector.tensor_copy(dst[:, 512 * sup:512 * (sup + 1)], pq[:])

            attn_outT = [aoT_pool.tile([128, N], BF16, tag=f"ao{p}", name=f"ao{p}")
                         for p in range(4)]

            def proj_step(nt):
                for oc in (0, 1):
                    pp = psq.tile([128, 512], F32, tag="qa", name="pp")
                    for ac in range(4):
                        nc.tensor.matmul(
                            pp[:],
                            attn_outT[ac][:, 128 * nt:128 * (nt + 1)],
                            pj_all[:, ac, 512 * oc:512 * (oc + 1)],
                            start=(ac == 0),
                            stop=(ac == 3),
                        )
                    ob = ob_pool.tile([128, 512], F32, tag="ob", name="ob")
                    nc.vector.tensor_copy(ob[:], pp[:])
                    nc.sync.dma_start(
                        out_d[128 * nt:128 * (nt + 1), 512 * oc:512 * (oc + 1)],
                        ob[:],
                    )

            # super-3 proj is split so only the pair-3 contribution remains
            # after the last attention super (short tail)
            pproj_sb = consts.tile([128, 4, 2, 512], F32, tag="pproj",
                                   name="pproj")

            def proj_partial(nt):
                for oc in (0, 1):
                    pp = psq.tile([128, 512], F32, tag="qa", name="pp")
                    for ac in range(3):
                        nc.tensor.matmul(
                            pp[:],
                            attn_outT[ac][:, 128 * nt:128 * (nt + 1)],
                            pj_all[:, ac, 512 * oc:512 * (oc + 1)],
                            start=(ac == 0),
                            stop=(ac == 2),
                        )
                    nc.vector.tensor_copy(pproj_sb[:, nt - 12, oc, :], pp[:])

            def proj_tail_step(nt, pool, tag):
                # full proj for one token tile with zero Vector involvement:
                # both oc chains into one 1024-wide psum, evacuated on the
                # (tail-idle) Scalar engine
                pp = pool.tile([128, 1024], F32, tag=tag, name="ppt")
                for oc in (0, 1):
                    for ac in range(4):
                        nc.tensor.matmul(
                            pp[:, 512 * oc:512 * (oc + 1)],
                            attn_outT[ac][:, 128 * nt:128 * (nt + 1)],
                            pj_all[:, ac, 512 * oc:512 * (oc + 1)],
                            start=(ac == 0),
                            stop=(ac == 3),
                        )
                ob = ob_pool.tile([128, 1024], F32, tag="ob2", name="obt")
                nc.scalar.copy(ob[:], pp[:])
                nc.sync.dma_start(out_d[128 * nt:128 * (nt + 1), :], ob[:])

            def proj_final(nt, pool, tag):
                pp = pool.tile([128, 1024], F32, tag=tag, name="pp")
                for oc in (0, 1):
                    nc.tensor.matmul(
                        pp[:, 512 * oc:512 * (oc + 1)],
                        attn_outT[3][:, 128 * nt:128 * (nt + 1)],
                        pj_all[:, 3, 512 * oc:512 * (oc + 1)],
                    )
                ob = ob_pool.tile([128, 1024], F32, tag="ob2", name="ob2")
                nc.vector.tensor_add(
                    ob[:], pp[:],
                    pproj_sb[:, nt - 12, :, :].rearrange("p a q -> p (a q)"),
                )
                nc.sync.dma_start(out_d[128 * nt:128 * (nt + 1), :], ob[:])

            # ---------------- filler machinery ----------------
            pending = []
            state = {}

            def fill(n):
                done = 0
                while pending and done < n:
                    kind, fn = pending.pop(0)
                    if kind == "w":
                        fn()
                    else:
                        fn()
                        done += 1

            def fill_all():
                while pending:
                    fill(4)

            def queue_pair(p):
                """Queue q/k chains for pair p (weights + 8 chains)."""
                qT = qk_pool.tile([128, N], BF16, tag="qk", name=f"q{p}")
                kT = qk_pool.tile([128, N], BF16, tag="qk", name=f"k{p}")

                def _wq():
                    state[f"wq{p}"] = load_w(p)

                def _wk():
                    state[f"wk{p}"] = load_w(4 + p)

                steps = [("w", _wq), ("w", _wk)]
                for sup in range(NS):
                    for key, dst in ((f"wq{p}", qT), (f"wk{p}", kT)):
                        def _c(key=key, dst=dst, sup=sup):
                            qk_chain(state[key], dst, sup)
                        steps.append(("q", _c))
                return qT, kT, steps

            # ---------------- prologue (q/k first: less DMA to wait on) ----
            qT = qk_pool.tile([128, N], BF16, tag="qk", name="q0")
            kT = qk_pool.tile([128, N], BF16, tag="qk", name="k0")
            qk_chain(wq0, qT, 0)
            qk_chain(wk0, kT, 0)
            for nt in range(4):
                v_chain(nt)
            for sup in range(1, NS):
                for wt, dst in ((wq0, qT), (wk0, kT)):
                    def _c(wt=wt, dst=dst, sup=sup):
                        qk_chain(wt, dst, sup)
                    pending.append(("q", _c))

            # ---------------- attention pair loop ----------------
            tail_fill = []
            deferred = []
            for p in range(4):
                if p < 3:
                    nq, nk, nsteps = queue_pair(p + 1)
                    pending.extend(nsteps)

                for s in range(NS):
                    if p == 0 and s > 0:
                        for nt in range(4 * s, 4 * s + 4):
                            v_chain(nt)
                    nkb = 4 * (s + 1)
                    oT = psO.tile([128, 1024], F32, tag="oT", name="oT")
                    P_prev = None
                    def pv_pair(kp, P):
                        # diagonal tiles only touch queries >= 128*r (causal)
                        q0 = 128 * max(kp - 4 * s, 0)
                        for h in (0, 1):
                            c0 = 512 * h
                            nc.tensor.matmul(
                                oT[:, c0 + q0:c0 + 512],
                                ve_all[:, kp, 2 * p + h, :],
                                P[:, c0 + q0:c0 + 512],
                                start=(kp == 0),
                                stop=(kp == nkb - 1),
                            )

                    ran_mid = False
                    for k in range(nkb):
                        if k == 2 and deferred:
                            deferred[0][0]()
                            ran_mid = True
                        if k == 5 and ran_mid and deferred:
                            deferred.pop(0)[1]()
                        if pending and (k % (4 if p < 3 else 2) == 0):
                            fill(1)
                        r = k - 4 * s
                        q0 = 128 * max(r, 0)
                        S2 = psS.tile([128, 1024], F32, tag="s2", name="S2")
                        for h in (0, 1):
                            hh = slice(64 * h, 64 * (h + 1))
                            nc.tensor.matmul(
                                S2[:, 512 * h + q0:512 * (h + 1)],
                                kT[hh, 128 * k:128 * (k + 1)],
                                qT[hh, 512 * s + q0:512 * (s + 1)],
                            )
                        P2 = pt_pool.tile([128, 1024], BF16, tag="pt", name="P2")
                        s3 = S2[:].rearrange("p (h q) -> p h q", h=2)
                        p3 = P2[:].rearrange("p (h q) -> p h q", h=2)
                        nc.scalar.activation(
                            p3[:, :, q0:512], s3[:, :, q0:512], EXP,
                            scale=float(D) ** -0.5,
                        )
                        if r >= 0:
                            for h in (0, 1):
                                c0 = 512 * h
                                nc.vector.tensor_mul(
                                    P2[:, c0 + q0:c0 + q0 + 128],
                                    P2[:, c0 + q0:c0 + q0 + 128],
                                    tril_sb[:],
                                )
                        if P_prev is not None:
                            pv_pair(k - 1, P_prev)
                        P_prev = P2
                    pv_pair(nkb - 1, P_prev)
                    if ran_mid and deferred:  # short super: stage2 after loop
                        deferred.pop(0)[1]()
                    last = p == 3 and s == NS - 1
                    if not last:
                        # evacuate psum fast; recip+broadcast and mul+DMA are
                        # deferred into the next super (two stages) so the
                        # Vector FIFO never head-of-line blocks on them
                        oTs = oTs_pool.tile([128, 1024], F32, tag="oTs",
                                            name="oTs")
                        nc.vector.tensor_copy(oTs[:], oT[:])
                        bcs = bcs_pool.tile([128, 1024], F32, tag="bcs",
                                            name="bcs")

                        def _norm_mid(oTs=oTs, bcs=bcs):
                            Rf = rf_pool.tile([1, 1024], F32, tag="rf",
                                              name="Rf")
                            nc.vector.reciprocal_approx_fast(Rf[:], oTs[0:1, :])
                            nc.gpsimd.partition_broadcast(bcs[:], Rf[:])

                        def _norm_tail(p=p, s=s, oTs=oTs, bcs=bcs):
                            tmp = tmp_pool.tile([128, 1024], BF16, tag="tmp",
                                                name="tmp")
                            nc.vector.tensor_mul(
                                tmp[64:128, :], oTs[64:128, :], bcs[64:128, :]
                            )
                            for h in (0, 1):
                                nc.sync.dma_start(
                                    attn_outT[p][64 * h:64 * (h + 1),
                                                 512 * s:512 * (s + 1)],
                                    tmp[64:128, 512 * h:512 * (h + 1)],
                                )
                            if p == 2 and s == NS - 1:
                                # ao[0..2] complete: super-3 partial proj
                                for nt in range(12, 16):
                                    def _pp(nt=nt):
                                        proj_partial(nt)
                                    pending.append(("p", _pp))
                            if p == 3 and s < NS - 1:
                                for nt in range(4 * s, 4 * s + 4):
                                    if s == 2 and nt >= 10:
                                        def _pt(nt=nt):
                                            proj_tail_step(nt, psS, "s2")
                                        tail_fill.append(_pt)
                                    else:
                                        def _pj(nt=nt):
                                            proj_step(nt)
                                        pending.append(("p", _pj))
                        deferred.append((_norm_mid, _norm_tail))
                    else:
                        # last super: psum-direct normalize in 128-token
                        # chunks; reserved proj steps (ScalarE evacuation, no
                        # Vector deps) keep the PE busy through the chain
                        if tail_fill:
                            tail_fill.pop(0)()
                        Rf = rf_pool.tile([1, 1024], F32, tag="rf", name="Rf")
                        nc.vector.reciprocal_approx_fast(Rf[:], oT[0:1, :])
                        bcs = bcs_pool.tile([128, 1024], F32, tag="bcs",
                                            name="bcs")
                        oT3 = oT[:].rearrange("p (h q) -> p h q", h=2)
                        Rf3 = Rf[:].rearrange("o (h q) -> o h q", h=2)
                        bc3 = bcs[:].rearrange("p (h q) -> p h q", h=2)
                        tmp = tmp_pool.tile([128, 1024], BF16, tag="tmp",
                                            name="tmp")
                        t3 = tmp[:].rearrange("p (h q) -> p h q", h=2)
                        for i, nt in enumerate(range(12, 16)):
                            cs = slice(128 * i, 128 * (i + 1))
                            nc.gpsimd.partition_broadcast(
                                bc3[:, :, cs], Rf3[:, :, cs]
                            )
                            nc.vector.tensor_mul(
                                t3[64:128, :, cs], oT3[64:128, :, cs],
                                bc3[64:128, :, cs]
                            )
                            for h in (0, 1):
                                nc.sync.dma_start(
                                    attn_outT[3][64 * h:64 * (h + 1),
                                                 1536 + 128 * i:1664 + 128 * i],
                                    tmp[64:128, 512 * h + 128 * i:
                                        512 * h + 128 * (i + 1)],
                                )
                            if i == 0 and tail_fill:
                                # before the first final: in-order PE would
                                # block it behind F12's DMA wait otherwise
                                tail_fill.pop(0)()
                            proj_final(nt, psS, "s2")
                if p < 3:
                    qT, kT = nq, nk
            for fn in tail_fill:
                fn()
            fill_all()

    nc.compile()
    return nc


def _tril_np():
    import ml_dtypes

    i = np.arange(128)[:, None]
    j = np.arange(128)[None, :]
    return (j >= i).astype(np.float32).astype(ml_dtypes.bfloat16)


def make_in_maps(x, qkv_w, proj_w):
    import ml_dtypes

    bf16 = ml_dtypes.bfloat16
    x = np.asarray(x, dtype=np.float32)
    qkv_w = np.asarray(qkv_w, dtype=np.float32)
    proj_w = np.asarray(proj_w, dtype=np.float32)
    tril = _tril_np()
    in_maps = []
    for c in range(8):
        b, g = c // 2, c % 2
        sl = slice(g * GC, (g + 1) * GC)
        wq, wk, wv = qkv_w[0:C][sl], qkv_w[C:2 * C][sl], qkv_w[2 * C:3 * C][sl]
        in_maps.append(
            {
                "xT": np.ascontiguousarray(x[b].T).astype(bf16),
                "wqkvT": np.ascontiguousarray(
                    np.concatenate([wq, wk, wv], 0).T
                ).astype(bf16),
                "projT": np.ascontiguousarray(proj_w[:, sl].T).astype(bf16),
                "tril": tril,
            }
        )
    return in_maps


def kernel(x, qkv_w, proj_w, proj_b):
    proj_b = np.asarray(proj_b, dtype=np.float32)

    if "nc" not in _cache:
        _cache["nc"] = _build_nc()
    nc = _cache["nc"]

    in_maps = make_in_maps(x, qkv_w, proj_w)
    res = run_bass_kernel_spmd(nc, in_maps, core_ids=list(range(8)))
    out = np.stack(
        [res.results[2 * b]["out"] + res.results[2 * b + 1]["out"] for b in range(B)], 0
    )
    return (out + proj_b[None, None, :]).astype(np.float32)


# revision 48
# speedup vs baseline: 1.0281x; 1.0140x over previous
"""Causal attention layer (B=4, N=2048, C=1024, H=16, D=64) on 8 TRN2 NeuronCores.

Sharding: core c -> (batch b = c//2, head-group g = c%2 of 8 heads).
All matmul operands are bf16 (fp32 lowers to two-pass fp32_mode=HIGH matmuls at
~1.5x the cost and defeats fast-weight-load).

Per core, for its (b, g), heads are processed as 4 pairs p (heads 2p, 2p+1):
  qT/kT[pair]  = wT_pair.T-contract(x)          [128 chan, N tok] bf16
  v            = x.T-contract(wv)               token-major [128 tok, 512 chan]
                 copied straight into ve_all[key, nt, head, 64:128] (no PE
                 transposes; col 0 of each 128-block holds ones for denominators)
  S2[k-tile]   = [kT[h0].T@qT[h0] | kT[h1].T@qT[h1]]   two K=64 matmuls on
                 disjoint PE row groups (tile_position (0,0)/(64,0)) -> overlap
  P2           = exp(S2 * D^-0.5) bf16, causal-masked on diagonal tiles; S/exp/
                 PV are restricted to the valid causal column range there
  oT[:,512h]  += ve_all[:,k,2p+h,:].T @ P2[:,512h]   row0 = denom, rows 64:128 = out
  attn_outT    = oT[64:128] * bcast(1/oT[0])    (recip + gpsimd partition_broadcast)
  out_part     = attn_outT.T-contract(projT)
Host sums the two head-group partials per batch and adds proj_b.

Scheduling notes (engine FIFOs are in-order, so emission order is everything):
- QKV chains for pair p+1, v chains, and the output projection are queued as
  fillers and paced into the attention stream so the PE stays dense while
  ScalarE runs exp (ScalarE is the pacer in attention-only stretches).
- Each super's softmax-normalize is split: psum evacuation at super end;
  recip+broadcast two iterations into the next super; mul+DMA three iterations
  later. Emitting them eagerly head-of-line blocks the Vector FIFO on the
  gpsimd broadcast, stalling the tril-mask muls that gate the PV matmuls.
- The final super's projection is split ac0-2/ac3 (partials run early as
  fillers), the last normalize+proj runs in 128-token chunks, and two reserved
  proj steps evacuate through the tail-idle Scalar engine so the tail has PE
  work with no Vector dependencies.
"""
import sys

sys.path.insert(0, "/opt/trn_rl_repo")

import numpy as np

import concourse.bass as bass  # noqa: F401
import concourse.tile as tile
from concourse import bacc, mybir
from concourse.bass_utils import run_bass_kernel_spmd

F32 = mybir.dt.float32
BF16 = mybir.dt.bfloat16
EXP = mybir.ActivationFunctionType.Exp

B, N, C, H, D = 4, 2048, 1024, 16, 64
G = 8            # heads per core
GC = G * D       # 512 channels per core
NT = N // 128    # 16 row tiles
NS = N // 512    # 4 row supers
CK = C // 128    # 8 contraction chunks

_cache = {}


def _build_nc():
    from contextlib import ExitStack

    nc = bacc.Bacc("TRN2", target_bir_lowering=False, debug=False)

    xT_d = nc.dram_tensor("xT", [C, N], BF16, kind="ExternalInput")
    wqkvT_d = nc.dram_tensor("wqkvT", [C, 3 * GC], BF16, kind="ExternalInput")
    projT_d = nc.dram_tensor("projT", [GC, C], BF16, kind="ExternalInput")
    tril_d = nc.dram_tensor("tril", [128, 128], BF16, kind="ExternalInput")
    out_d = nc.dram_tensor("out", [N, C], F32, kind="ExternalOutput")

    with tile.TileContext(nc) as tc:
        with ExitStack() as ctx:
            consts = ctx.enter_context(tc.tile_pool(name="consts", bufs=1))
            qk_pool = ctx.enter_context(tc.tile_pool(name="qk", bufs=4))
            ve_pool = ctx.enter_context(tc.tile_pool(name="ve", bufs=1))
            wv_pool = ctx.enter_context(tc.tile_pool(name="wv", bufs=1))
            w_pool = ctx.enter_context(tc.tile_pool(name="wA", bufs=4))
            xT_pool = ctx.enter_context(tc.tile_pool(name="xT", bufs=1))
            rf_pool = ctx.enter_context(tc.tile_pool(name="rf", bufs=2))
            bcs_pool = ctx.enter_context(tc.tile_pool(name="bcs", bufs=2))
            tmp_pool = ctx.enter_context(tc.tile_pool(name="tmp", bufs=2))
            oTs_pool = ctx.enter_context(tc.tile_pool(name="oTs", bufs=2))
            ob_pool = ctx.enter_context(tc.tile_pool(name="ob", bufs=2))
            pj_pool = ctx.enter_context(tc.tile_pool(name="pj", bufs=1))
            aoT_pool = ctx.enter_context(tc.tile_pool(name="aoT", bufs=1))
            pt_pool = ctx.enter_context(tc.tile_pool(name="pt", bufs=4))
            psq = ctx.enter_context(tc.tile_pool(name="psq", bufs=2, space="PSUM"))
            psS = ctx.enter_context(tc.tile_pool(name="psS", bufs=2, space="PSUM"))
            psO = ctx.enter_context(tc.tile_pool(name="psO", bufs=1, space="PSUM"))

            tril_sb = consts.tile([128, 128], BF16)
            nc.sync.dma_start(tril_sb[:], tril_d[:])

            wv_sb = wv_pool.tile([128, CK, GC], BF16, name="wv")

            def load_w(ot):
                wt = w_pool.tile([128, CK, 128], BF16, tag="wt", name=f"wt{ot}")
                src = wqkvT_d[:, 128 * ot:128 * (ot + 1)].rearrange(
                    "(cc p) o -> p cc o", p=128
                )
                nc.sync.dma_start(wt[:], src)
                return wt

            def x_sup_dma(sup):
                nc.sync.dma_start(
                    xs_all[:, :, 512 * sup:512 * (sup + 1)],
                    xT_d[:, 512 * sup:512 * (sup + 1)].rearrange(
                        "(cc p) n -> p cc n", p=128
                    ),
                )

            # one big DMA per x super (fans out across the 16 HW queues);
            # pair-0 weights first so the first q/k chain can start ASAP
            xs_all = xT_pool.tile([128, CK, N], BF16, name="xs_all")
            wq0 = load_w(0)
            for qtr in range(4):
                nc.sync.dma_start(
                    xs_all[:, 2 * qtr:2 * (qtr + 1), 0:512],
                    xT_d[256 * qtr:256 * (qtr + 1), 0:512].rearrange(
                        "(cc p) n -> p cc n", p=128
                    ),
                )
            wk0 = load_w(4)
            for half in (0, 1):
                nc.sync.dma_start(
                    wv_sb[:, 4 * half:4 * (half + 1), :],
                    wqkvT_d[512 * half:512 * (half + 1), 2 * GC:3 * GC].rearrange(
                        "(cc p) o -> p cc o", p=128
                    ),
                )
            for sup in range(1, NS):
                x_sup_dma(sup)

            pj_all = pj_pool.tile([128, 4, C], BF16, name="pj_all")
            nc.sync.dma_start(
                pj_all[:], projT_d[:].rearrange("(ac p) o -> p ac o", p=128)
            )

            # ve_all[key, nt, head, col]: col 0 = ones, cols 64:128 = v
            ve_all = ve_pool.tile([128, NT, G, 128], BF16, name="ve")
            nc.vector.memset(ve_all[:, :, :, 0:1], 1.0)

            def v_chain(nt):
                psv = psq.tile([128, GC], F32, tag="qa", name=f"psv{nt}")
                for cc in range(CK):
                    nc.tensor.matmul(
                        psv[:],
                        xs_all[:, cc, 128 * nt:128 * (nt + 1)],
                        wv_sb[:, cc, :],
                        start=(cc == 0),
                        stop=(cc == CK - 1),
                    )
                nc.vector.tensor_copy(
                    ve_all[:, nt, :, 64:128],
                    psv[:].rearrange("p (h d) -> p h d", h=G),
                )

            def qk_chain(wt, dst, sup):
                pq = psq.tile([128, 512], F32, tag="qa", name="pq")
                for cc in range(CK):
                    nc.tensor.matmul(
                        pq[:],
                        wt[:, cc, :],
                        xs_all[:, cc, 512 * sup:512 * (sup + 1)],
                        start=(cc == 0),
                        stop=(cc == CK - 1),
                    )
                nc.vector.tensor_copy(dst[:, 512 * sup:512 * (sup + 1)], pq[:])

            attn_outT = [aoT_pool.tile([128, N], BF16, tag=f"ao{p}", name=f"ao{p}")
                         for p in range(4)]

            def proj_step(nt):
                for oc in (0, 1):
                    pp = psq.tile([128, 512], F32, tag="qa", name="pp")
                    for ac in range(4):
                        nc.tensor.matmul(
                            pp[:],
                            attn_outT[ac][:, 128 * nt:128 * (nt + 1)],
                            pj_all[:, ac, 512 * oc:512 * (oc + 1)],
                            start=(ac == 0),
                            stop=(ac == 3),
                        )
                    ob = ob_pool.tile([128, 512], F32, tag="ob", name="ob")
                    nc.vector.tensor_copy(ob[:], pp[:])
                    nc.sync.dma_start(
                        out_d[128 * nt:128 * (nt + 1), 512 * oc:512 * (oc + 1)],
                        ob[:],
                    )

            # super-3 proj is split so only the pair-3 contribution remains
            # after the last attention super (short tail)
            pproj_sb = consts.tile([128, 4, 2, 512], F32, tag="pproj",
                                   name="pproj")

            def proj_partial(nt):
                for oc in (0, 1):
                    pp = psq.tile([128, 512], F32, tag="qa", name="pp")
                    for ac in range(3):
                        nc.tensor.matmul(
                            pp[:],
                            attn_outT[ac][:, 128 * nt:128 * (nt + 1)],
                            pj_all[:, ac, 512 * oc:512 * (oc + 1)],
                            start=(ac == 0),
                            stop=(ac == 2),
                        )
                    nc.vector.tensor_copy(pproj_sb[:, nt - 12, oc, :], pp[:])

            def proj_tail_step(nt, pool, tag):
                # full proj for one token tile with zero Vector involvement:
                # both oc chains into one 1024-wide psum, evacuated on the
                # (tail-idle) Scalar engine
                pp = pool.tile([128, 1024], F32, tag=tag, name="ppt")
                for oc in (0, 1):
                    for ac in range(4):
                        nc.tensor.matmul(
                            pp[:, 512 * oc:512 * (oc + 1)],
                            attn_outT[ac][:, 128 * nt:128 * (nt + 1)],
                            pj_all[:, ac, 512 * oc:512 * (oc + 1)],
                            start=(ac == 0),
                            stop=(ac == 3),
                        )
                ob = ob_pool.tile([128, 1024], F32, tag="ob2", name="obt")
                nc.scalar.copy(ob[:], pp[:])
                nc.sync.dma_start(out_d[128 * nt:128 * (nt + 1), :], ob[:])

            def proj_final(nt, pool, tag):
                pp = pool.tile([128, 1024], F32, tag=tag, name="pp")
                for oc in (0, 1):
                    nc.tensor.matmul(
                        pp[:, 512 * oc:512 * (oc + 1)],
                        attn_outT[3][:, 128 * nt:128 * (nt + 1)],
                        pj_all[:, 3, 512 * oc:512 * (oc + 1)],
                    )
                ob = ob_pool.tile([128, 1024], F32, tag="ob2", name="ob2")
                nc.vector.tensor_add(
                    ob[:], pp[:],
                    pproj_sb[:, nt - 12, :, :].rearrange("p a q -> p (a q)"),
                )
                nc.sync.dma_start(out_d[128 * nt:128 * (nt + 1), :], ob[:])

            # ---------------- filler machinery ----------------
            pending = []
            state = {}

            def fill(n):
                done = 0
                while pending and done < n:
                    kind, fn = pending.pop(0)
                    if kind == "w":
                        fn()
                    else:
                        fn()
                        done += 1

            def fill_all():
                while pending:
                    fill(4)

            def queue_pair(p):
                """Queue q/k chains for pair p (weights + 8 chains)."""
                qT = qk_pool.tile([128, N], BF16, tag="qk", name=f"q{p}")
                kT = qk_pool.tile([128, N], BF16, tag="qk", name=f"k{p}")

                def _wq():
                    state[f"wq{p}"] = load_w(p)

                def _wk():
                    state[f"wk{p}"] = load_w(4 + p)

                steps = [("w", _wq), ("w", _wk)]
                for sup in range(NS):
                    for key, dst in ((f"wq{p}", qT), (f"wk{p}", kT)):
                        def _c(key=key, dst=dst, sup=sup):
                            qk_chain(state[key], dst, sup)
                        steps.append(("q", _c))
                return qT, kT, steps

            # ---------------- prologue (q/k first: less DMA to wait on) ----
            qT = qk_pool.tile([128, N], BF16, tag="qk", name="q0")
            kT = qk_pool.tile([128, N], BF16, tag="qk", name="k0")
            qk_chain(wq0, qT, 0)
            qk_chain(wk0, kT, 0)
            for nt in range(4):
                v_chain(nt)
            for sup in range(1, NS):
                for wt, dst in ((wq0, qT), (wk0, kT)):
                    def _c(wt=wt, dst=dst, sup=sup):
                        qk_chain(wt, dst, sup)
                    pending.append(("q", _c))

            # ---------------- attention pair loop ----------------
            tail_fill = []
            deferred = []
            for p in range(4):
                if p < 3:
                    nq, nk, nsteps = queue_pair(p + 1)
                    pending.extend(nsteps)

                for s in range(NS):
                    if p == 0 and s > 0:
                        for nt in range(4 * s, 4 * s + 4):
                            v_chain(nt)
                    nkb = 4 * (s + 1)
                    oT = psO.tile([128, 1024], F32, tag="oT", name="oT")
                    P_prev = None
                    def pv_pair(kp, P):
                        # diagonal tiles only touch queries >= 128*r (causal)
                        q0 = 128 * max(kp - 4 * s, 0)
                        for h in (0, 1):
                            c0 = 512 * h
                            nc.tensor.matmul(
                                oT[:, c0 + q0:c0 + 512],
                                ve_all[:, kp, 2 * p + h, :],
                                P[:, c0 + q0:c0 + 512],
                                start=(kp == 0),
                                stop=(kp == nkb - 1),
                            )

                    ran_mid = False
                    for k in range(nkb):
                        if k == 2 and deferred:
                            deferred[0][0]()
                            ran_mid = True
                        if k == 5 and ran_mid and deferred:
                            deferred.pop(0)[1]()
                        if pending and (k % (4 if p < 3 else 2) == 0):
                            fill(1)
                        r = k - 4 * s
                        q0 = 128 * max(r, 0)
                        S2 = psS.tile([128, 1024], F32, tag="s2", name="S2")
                        for h in (0, 1):
                            hh = slice(64 * h, 64 * (h + 1))
                            nc.tensor.matmul(
                                S2[:, 512 * h + q0:512 * (h + 1)],
                                kT[hh, 128 * k:128 * (k + 1)],
                                qT[hh, 512 * s + q0:512 * (s + 1)],
                            )
                        P2 = pt_pool.tile([128, 1024], BF16, tag="pt", name="P2")
                        s3 = S2[:].rearrange("p (h q) -> p h q", h=2)
                        p3 = P2[:].rearrange("p (h q) -> p h q", h=2)
                        nc.scalar.activation(
                            p3[:, :, q0:512], s3[:, :, q0:512], EXP,
                            scale=float(D) ** -0.5,
                        )
                        if r >= 0:
                            for h in (0, 1):
                                c0 = 512 * h
                                nc.vector.tensor_mul(
                                    P2[:, c0 + q0:c0 + q0 + 128],
                                    P2[:, c0 + q0:c0 + q0 + 128],
                                    tril_sb[:],
                                )
                        if P_prev is not None:
                            pv_pair(k - 1, P_prev)
                        P_prev = P2
                    pv_pair(nkb - 1, P_prev)
                    if ran_mid and deferred:  # short super: stage2 after loop
                        deferred.pop(0)[1]()
                    last = p == 3 and s == NS - 1
                    if not last:
                        # evacuate psum fast; recip+broadcast and mul+DMA are
                        # deferred into the next super (two stages) so the
                        # Vector FIFO never head-of-line blocks on them
                        oTs = oTs_pool.tile([128, 1024], F32, tag="oTs",
                                            name="oTs")
                        nc.vector.tensor_copy(oTs[:], oT[:])
                        bcs = bcs_pool.tile([128, 1024], F32, tag="bcs",
                                            name="bcs")

                        def _norm_mid(oTs=oTs, bcs=bcs):
                            Rf = rf_pool.tile([1, 1024], F32, tag="rf",
                                              name="Rf")
                            nc.vector.reciprocal_approx_fast(Rf[:], oTs[0:1, :])
                            nc.gpsimd.partition_broadcast(bcs[:], Rf[:])

                        def _norm_tail(p=p, s=s, oTs=oTs, bcs=bcs):
                            tmp = tmp_pool.tile([128, 1024], BF16, tag="tmp",
                                                name="tmp")
                            nc.vector.tensor_mul(
                                tmp[64:128, :], oTs[64:128, :], bcs[64:128, :]
                            )
                            for h in (0, 1):
                                nc.sync.dma_start(
                                    attn_outT[p][64 * h:64 * (h + 1),
                                                 512 * s:512 * (s + 1)],
                                    tmp[64:128, 512 * h:512 * (h + 1)],
                                )
                            if p == 2 and s == NS - 1:
                                # ao[0..2] complete: super-3 partial proj
                                for nt in range(12, 16):
                                    def _pp(nt=nt):
                                        proj_partial(nt)
                                    pending.append(("p", _pp))
                            if p == 3 and s < NS - 1:
                                for nt in range(4 * s, 4 * s + 4):
                                    if s == 2 and nt >= 10:
                                        def _pt(nt=nt):
                                            proj_tail_step(nt, psS, "s2")
                                        tail_fill.append(_pt)
                                    else:
                                        def _pj(nt=nt):
                                            proj_step(nt)
                                        pending.append(("p", _pj))
                        deferred.append((_norm_mid, _norm_tail))
                    else:
                        # last super: psum-direct normalize in 128-token
                        # chunks; reserved proj steps (ScalarE evacuation, no
                        # Vector deps) keep the PE busy through the chain
                        if tail_fill:
                            tail_fill.pop(0)()
                        Rf = rf_pool.tile([1, 1024], F32, tag="rf", name="Rf")
                        nc.vector.reciprocal_approx_fast(Rf[:], oT[0:1, :])
                        bcs = bcs_pool.tile([128, 1024], F32, tag="bcs",
                                            name="bcs")
                        oT3 = oT[:].rearrange("p (h q) -> p h q", h=2)
                        Rf3 = Rf[:].rearrange("o (h q) -> o h q", h=2)
                        bc3 = bcs[:].rearrange("p (h q) -> p h q", h=2)
                        tmp = tmp_pool.tile([128, 1024], BF16, tag="tmp",
                                            name="tmp")
                        t3 = tmp[:].rearrange("p (h q) -> p h q", h=2)
                        for i, nt in enumerate(range(12, 16)):
                            cs = slice(128 * i, 128 * (i + 1))
                            nc.gpsimd.partition_broadcast(
                                bc3[:, :, cs], Rf3[:, :, cs]
                            )
                            nc.vector.tensor_mul(
                                t3[64:128, :, cs], oT3[64:128, :, cs],
                                bc3[64:128, :, cs]
                            )
                            for h in (0, 1):
                                nc.sync.dma_start(
                                    attn_outT[3][64 * h:64 * (h + 1),
                                                 1536 + 128 * i:1664 + 128 * i],
                                    tmp[64:128, 512 * h + 128 * i:
                                        512 * h + 128 * (i + 1)],
                                )
                            if i == 0 and tail_fill:
                                # before the first final: in-order PE would
                                # block it behind F12's DMA wait otherwise
                                tail_fill.pop(0)()
                            proj_final(nt, psS, "s2")
                if p < 3:
                    qT, kT = nq, nk
            for fn in tail_fill:
                fn()
            fill_all()

    nc.compile()
    return nc


def _tril_np():
    import ml_dtypes

    i = np.arange(128)[:, None]
    j = np.arange(128)[None, :]
    return (j >= i).astype(np.float32).astype(ml_dtypes.bfloat16)


def make_in_maps(x, qkv_w, proj_w):
    import ml_dtypes

    bf16 = ml_dtypes.bfloat16
    x = np.asarray(x, dtype=np.float32)
    qkv_w = np.asarray(qkv_w, dtype=np.float32)
    proj_w = np.asarray(proj_w, dtype=np.float32)
    tril = _tril_np()
    in_maps = []
    for c in range(8):
        b, g = c // 2, c % 2
        sl = slice(g * GC, (g + 1) * GC)
        wq, wk, wv = qkv_w[0:C][sl], qkv_w[C:2 * C][sl], qkv_w[2 * C:3 * C][sl]
        in_maps.append(
            {
                "xT": np.ascontiguousarray(x[b].T).astype(bf16),
                "wqkvT": np.ascontiguousarray(
                    np.concatenate([wq, wk, wv], 0).T
                ).astype(bf16),
                "projT": np.ascontiguousarray(proj_w[:, sl].T).astype(bf16),
                "tril": tril,
            }
        )
    return in_maps


def kernel(x, qkv_w, proj_w, proj_b):
    proj_b = np.asarray(proj_b, dtype=np.float32)

    if "nc" not in _cache:
        _cache["nc"] = _build_nc()
    nc = _cache["nc"]

    in_maps = make_in_maps(x, qkv_w, proj_w)
    res = run_bass_kernel_spmd(nc, in_maps, core_ids=list(range(8)))
    out = np.stack(
        [res.results[2 * b]["out"] + res.results[2 * b + 1]["out"] for b in range(B)], 0
    )
    return (out + proj_b[None, None, :]).astype(np.float32)


# revision 51
# speedup vs baseline: 1.0552x; 1.0263x over previous
"""Causal attention layer (B=4, N=2048, C=1024, H=16, D=64) on 8 TRN2 NeuronCores.

Sharding: core c -> (batch b = c//2, head-group g = c%2 of 8 heads).
All matmul operands are bf16 (fp32 lowers to two-pass fp32_mode=HIGH matmuls at
~1.5x the cost and defeats fast-weight-load).

Per core, for its (b, g), heads are processed as 4 pairs p (heads 2p, 2p+1):
  qT/kT[pair]  = wT_pair.T-contract(x)          [128 chan, N tok] bf16
  v            = x.T-contract(wv)               token-major [128 tok, 512 chan]
                 copied straight into ve_all[key, nt, head, 64:128] (no PE
                 transposes; col 0 of each 128-block holds ones for denominators)
  S2[k-tile]   = [kT[h0].T@qT[h0] | kT[h1].T@qT[h1]]   two K=64 matmuls on
                 disjoint PE row groups (tile_position (0,0)/(64,0)) -> overlap
  P2           = exp(S2 * D^-0.5) bf16, causal-masked on diagonal tiles; S/exp/
                 PV are restricted to the valid causal column range there
  oT[:,512h]  += ve_all[:,k,2p+h,:].T @ P2[:,512h]   row0 = denom, rows 64:128 = out
  attn_outT    = oT[64:128] * bcast(1/oT[0])    (recip + gpsimd partition_broadcast)
  out_part     = attn_outT.T-contract(projT)
Host sums the two head-group partials per batch and adds proj_b.

Scheduling notes (engine FIFOs are in-order, so emission order is everything):
- QKV chains for pair p+1, v chains, and the output projection are queued as
  fillers and paced into the attention stream so the PE stays dense while
  ScalarE runs exp (ScalarE is the pacer in attention-only stretches).
- Each super's softmax-normalize is split: psum evacuation at super end;
  recip+broadcast two iterations into the next super; mul+DMA three iterations
  later. Emitting them eagerly head-of-line blocks the Vector FIFO on the
  gpsimd broadcast, stalling the tril-mask muls that gate the PV matmuls.
- The final super's projection is split ac0-2/ac3 (partials run early as
  fillers), the last normalize+proj runs in 128-token chunks, and two reserved
  proj steps evacuate through the tail-idle Scalar engine so the tail has PE
  work with no Vector dependencies.
"""
import sys

sys.path.insert(0, "/opt/trn_rl_repo")

import numpy as np

import concourse.bass as bass  # noqa: F401
import concourse.tile as tile
from concourse import bacc, mybir
from concourse.bass_utils import run_bass_kernel_spmd

F32 = mybir.dt.float32
BF16 = mybir.dt.bfloat16
EXP = mybir.ActivationFunctionType.Exp

B, N, C, H, D = 4, 2048, 1024, 16, 64
G = 8            # heads per core
GC = G * D       # 512 channels per core
NT = N // 128    # 16 row tiles
NS = N // 512    # 4 row supers
CK = C // 128    # 8 contraction chunks

_cache = {}


def _build_nc():
    from contextlib import ExitStack

    nc = bacc.Bacc("TRN2", target_bir_lowering=False, debug=False)

    xT_d = nc.dram_tensor("xT", [C, N], BF16, kind="ExternalInput")
    wqkvT_d = nc.dram_tensor("wqkvT", [C, 3 * GC], BF16, kind="ExternalInput")
    projT_d = nc.dram_tensor("projT", [GC, C], BF16, kind="ExternalInput")
    tril_d = nc.dram_tensor("tril", [128, 128], BF16, kind="ExternalInput")
    out_d = nc.dram_tensor("out", [N, C], F32, kind="ExternalOutput")

    with tile.TileContext(nc) as tc:
        with ExitStack() as ctx:
            consts = ctx.enter_context(tc.tile_pool(name="consts", bufs=1))
            qk_pool = ctx.enter_context(tc.tile_pool(name="qk", bufs=4))
            ve_pool = ctx.enter_context(tc.tile_pool(name="ve", bufs=1))
            wv_pool = ctx.enter_context(tc.tile_pool(name="wv", bufs=1))
            w_pool = ctx.enter_context(tc.tile_pool(name="wA", bufs=4))
            xT_pool = ctx.enter_context(tc.tile_pool(name="xT", bufs=1))
            rf_pool = ctx.enter_context(tc.tile_pool(name="rf", bufs=2))
            bcs_pool = ctx.enter_context(tc.tile_pool(name="bcs", bufs=2))
            tmp_pool = ctx.enter_context(tc.tile_pool(name="tmp", bufs=2))
            oTs_pool = ctx.enter_context(tc.tile_pool(name="oTs", bufs=2))
            ob_pool = ctx.enter_context(tc.tile_pool(name="ob", bufs=2))
            pj_pool = ctx.enter_context(tc.tile_pool(name="pj", bufs=1))
            aoT_pool = ctx.enter_context(tc.tile_pool(name="aoT", bufs=1))
            pt_pool = ctx.enter_context(tc.tile_pool(name="pt", bufs=5))
            psq = ctx.enter_context(tc.tile_pool(name="psq", bufs=2, space="PSUM"))
            psS = ctx.enter_context(tc.tile_pool(name="psS", bufs=2, space="PSUM"))
            psO = ctx.enter_context(tc.tile_pool(name="psO", bufs=1, space="PSUM"))

            tril_sb = consts.tile([128, 128], BF16)
            nc.sync.dma_start(tril_sb[:], tril_d[:])

            wv_sb = wv_pool.tile([128, CK, GC], BF16, name="wv")

            def load_w(ot):
                wt = w_pool.tile([128, CK, 128], BF16, tag="wt", name=f"wt{ot}")
                src = wqkvT_d[:, 128 * ot:128 * (ot + 1)].rearrange(
                    "(cc p) o -> p cc o", p=128
                )
                nc.sync.dma_start(wt[:], src)
                return wt

            def x_sup_dma(sup):
                nc.sync.dma_start(
                    xs_all[:, :, 512 * sup:512 * (sup + 1)],
                    xT_d[:, 512 * sup:512 * (sup + 1)].rearrange(
                        "(cc p) n -> p cc n", p=128
                    ),
                )

            # one big DMA per x super (fans out across the 16 HW queues);
            # pair-0 weights first so the first q/k chain can start ASAP
            xs_all = xT_pool.tile([128, CK, N], BF16, name="xs_all")
            wq0 = load_w(0)
            for qtr in range(4):
                nc.sync.dma_start(
                    xs_all[:, 2 * qtr:2 * (qtr + 1), 0:512],
                    xT_d[256 * qtr:256 * (qtr + 1), 0:512].rearrange(
                        "(cc p) n -> p cc n", p=128
                    ),
                )
            wk0 = load_w(4)
            for half in (0, 1):
                nc.sync.dma_start(
                    wv_sb[:, 4 * half:4 * (half + 1), :],
                    wqkvT_d[512 * half:512 * (half + 1), 2 * GC:3 * GC].rearrange(
                        "(cc p) o -> p cc o", p=128
                    ),
                )
            for sup in range(1, NS):
                x_sup_dma(sup)

            pj_all = pj_pool.tile([128, 4, C], BF16, name="pj_all")
            nc.sync.dma_start(
                pj_all[:], projT_d[:].rearrange("(ac p) o -> p ac o", p=128)
            )

            # ve_all[key, nt, head, col]: col 0 = ones, cols 64:128 = v
            ve_all = ve_pool.tile([128, NT, G, 128], BF16, name="ve")
            nc.vector.memset(ve_all[:, :, :, 0:1], 1.0)

            def v_chain(nt):
                psv = psq.tile([128, GC], F32, tag="qa", name=f"psv{nt}")
                for cc in range(CK):
                    nc.tensor.matmul(
                        psv[:],
                        xs_all[:, cc, 128 * nt:128 * (nt + 1)],
                        wv_sb[:, cc, :],
                        start=(cc == 0),
                        stop=(cc == CK - 1),
                    )
                nc.vector.tensor_copy(
                    ve_all[:, nt, :, 64:128],
                    psv[:].rearrange("p (h d) -> p h d", h=G),
                )

            def qk_chain(wt, dst, sup):
                pq = psq.tile([128, 512], F32, tag="qa", name="pq")
                for cc in range(CK):
                    nc.tensor.matmul(
                        pq[:],
                        wt[:, cc, :],
                        xs_all[:, cc, 512 * sup:512 * (sup + 1)],
                        start=(cc == 0),
                        stop=(cc == CK - 1),
                    )
                nc.vector.tensor_copy(dst[:, 512 * sup:512 * (sup + 1)], pq[:])

            attn_outT = [aoT_pool.tile([128, N], BF16, tag=f"ao{p}", name=f"ao{p}")
                         for p in range(4)]

            def proj_step(nt):
                for oc in (0, 1):
                    pp = psq.tile([128, 512], F32, tag="qa", name="pp")
                    for ac in range(4):
                        nc.tensor.matmul(
                            pp[:],
                            attn_outT[ac][:, 128 * nt:128 * (nt + 1)],
                            pj_all[:, ac, 512 * oc:512 * (oc + 1)],
                            start=(ac == 0),
                            stop=(ac == 3),
                        )
                    ob = ob_pool.tile([128, 512], F32, tag="ob", name="ob")
                    nc.vector.tensor_copy(ob[:], pp[:])
                    nc.sync.dma_start(
                        out_d[128 * nt:128 * (nt + 1), 512 * oc:512 * (oc + 1)],
                        ob[:],
                    )

            # super-3 proj is split so only the pair-3 contribution remains
            # after the last attention super (short tail)
            pproj_sb = consts.tile([128, 4, 2, 512], F32, tag="pproj",
                                   name="pproj")

            def proj_partial(nt):
                for oc in (0, 1):
                    pp = psq.tile([128, 512], F32, tag="qa", name="pp")
                    for ac in range(3):
                        nc.tensor.matmul(
                            pp[:],
                            attn_outT[ac][:, 128 * nt:128 * (nt + 1)],
                            pj_all[:, ac, 512 * oc:512 * (oc + 1)],
                            start=(ac == 0),
                            stop=(ac == 2),
                        )
                    nc.vector.tensor_copy(pproj_sb[:, nt - 12, oc, :], pp[:])

            def proj_tail_step(nt, pool, tag):
                # full proj for one token tile with zero Vector involvement:
                # both oc chains into one 1024-wide psum, evacuated on the
                # (tail-idle) Scalar engine
                pp = pool.tile([128, 1024], F32, tag=tag, name="ppt")
                for oc in (0, 1):
                    for ac in range(4):
                        nc.tensor.matmul(
                            pp[:, 512 * oc:512 * (oc + 1)],
                            attn_outT[ac][:, 128 * nt:128 * (nt + 1)],
                            pj_all[:, ac, 512 * oc:512 * (oc + 1)],
                            start=(ac == 0),
                            stop=(ac == 3),
                        )
                ob = ob_pool.tile([128, 1024], F32, tag="ob2", name="obt")
                nc.scalar.copy(ob[:], pp[:])
                nc.sync.dma_start(out_d[128 * nt:128 * (nt + 1), :], ob[:])

            def proj_final(nt, pool, tag):
                pp = pool.tile([128, 1024], F32, tag=tag, name="pp")
                for oc in (0, 1):
                    nc.tensor.matmul(
                        pp[:, 512 * oc:512 * (oc + 1)],
                        attn_outT[3][:, 128 * nt:128 * (nt + 1)],
                        pj_all[:, 3, 512 * oc:512 * (oc + 1)],
                    )
                ob = ob_pool.tile([128, 1024], F32, tag="ob2", name="ob2")
                nc.vector.tensor_add(
                    ob[:], pp[:],
                    pproj_sb[:, nt - 12, :, :].rearrange("p a q -> p (a q)"),
                )
                nc.sync.dma_start(out_d[128 * nt:128 * (nt + 1), :], ob[:])

            # ---------------- filler machinery ----------------
            pending = []
            state = {}

            def fill(n):
                done = 0
                while pending and done < n:
                    kind, fn = pending.pop(0)
                    if kind == "w":
                        fn()
                    else:
                        fn()
                        done += 1

            def fill_all():
                while pending:
                    fill(4)

            def queue_pair(p):
                """Queue q/k chains for pair p (weights + 8 chains)."""
                qT = qk_pool.tile([128, N], BF16, tag="qk", name=f"q{p}")
                kT = qk_pool.tile([128, N], BF16, tag="qk", name=f"k{p}")

                def _wq():
                    state[f"wq{p}"] = load_w(p)

                def _wk():
                    state[f"wk{p}"] = load_w(4 + p)

                steps = [("w", _wq), ("w", _wk)]
                for sup in range(NS):
                    for key, dst in ((f"wq{p}", qT), (f"wk{p}", kT)):
                        def _c(key=key, dst=dst, sup=sup):
                            qk_chain(state[key], dst, sup)
                        steps.append(("q", _c))
                return qT, kT, steps

            # ---------------- prologue (q/k first: less DMA to wait on) ----
            qT = qk_pool.tile([128, N], BF16, tag="qk", name="q0")
            kT = qk_pool.tile([128, N], BF16, tag="qk", name="k0")
            qk_chain(wq0, qT, 0)
            qk_chain(wk0, kT, 0)
            for nt in range(4):
                v_chain(nt)
            for sup in range(1, NS):
                for wt, dst in ((wq0, qT), (wk0, kT)):
                    def _c(wt=wt, dst=dst, sup=sup):
                        qk_chain(wt, dst, sup)
                    pending.append(("q", _c))

            # ---------------- attention pair loop ----------------
            tail_fill = []
            deferred = []
            for p in range(4):
                if p < 3:
                    nq, nk, nsteps = queue_pair(p + 1)
                    pending.extend(nsteps)

                for s in range(NS):
                    if p == 0 and s > 0:
                        for nt in range(4 * s, 4 * s + 4):
                            v_chain(nt)
                    nkb = 4 * (s + 1)
                    oT = psO.tile([128, 1024], F32, tag="oT", name="oT")
                    P_hist = []
                    def pv_pair(kp, P):
                        # diagonal tiles only touch queries >= 128*r (causal)
                        q0 = 128 * max(kp - 4 * s, 0)
                        for h in (0, 1):
                            c0 = 512 * h
                            nc.tensor.matmul(
                                oT[:, c0 + q0:c0 + 512],
                                ve_all[:, kp, 2 * p + h, :],
                                P[:, c0 + q0:c0 + 512],
                                start=(kp == 0),
                                stop=(kp == nkb - 1),
                            )

                    ran_mid = False
                    for k in range(nkb):
                        if k == 2 and deferred:
                            deferred[0][0]()
                            ran_mid = True
                        if k == 5 and ran_mid and deferred:
                            deferred.pop(0)[1]()
                        if pending and (k % (4 if p < 3 else 2) == 0):
                            fill(1)
                        r = k - 4 * s
                        q0 = 128 * max(r, 0)
                        S2 = psS.tile([128, 1024], F32, tag="s2", name="S2")
                        for h in (0, 1):
                            hh = slice(64 * h, 64 * (h + 1))
                            nc.tensor.matmul(
                                S2[:, 512 * h + q0:512 * (h + 1)],
                                kT[hh, 128 * k:128 * (k + 1)],
                                qT[hh, 512 * s + q0:512 * (s + 1)],
                            )
                        P2 = pt_pool.tile([128, 1024], BF16, tag="pt", name="P2")
                        s3 = S2[:].rearrange("p (h q) -> p h q", h=2)
                        p3 = P2[:].rearrange("p (h q) -> p h q", h=2)
                        nc.scalar.activation(
                            p3[:, :, q0:512], s3[:, :, q0:512], EXP,
                            scale=float(D) ** -0.5,
                        )
                        if r >= 0:
                            for h in (0, 1):
                                c0 = 512 * h
                                nc.vector.tensor_mul(
                                    P2[:, c0 + q0:c0 + q0 + 128],
                                    P2[:, c0 + q0:c0 + q0 + 128],
                                    tril_sb[:],
                                )
                        P_hist.append(P2)
                        if len(P_hist) > 2:  # PV lags S by 2: exp+mask
                            pv_pair(k - 2, P_hist.pop(0))  # latency hidden
                    for j, Pj in enumerate(P_hist):
                        pv_pair(nkb - len(P_hist) + j, Pj)
                    if ran_mid and deferred:  # short super: stage2 after loop
                        deferred.pop(0)[1]()
                    last = p == 3 and s == NS - 1
                    if not last:
                        # evacuate psum fast; recip+broadcast and mul+DMA are
                        # deferred into the next super (two stages) so the
                        # Vector FIFO never head-of-line blocks on them
                        oTs = oTs_pool.tile([128, 1024], F32, tag="oTs",
                                            name="oTs")
                        nc.vector.tensor_copy(oTs[:], oT[:])
                        bcs = bcs_pool.tile([128, 1024], F32, tag="bcs",
                                            name="bcs")

                        def _norm_mid(oTs=oTs, bcs=bcs):
                            Rf = rf_pool.tile([1, 1024], F32, tag="rf",
                                              name="Rf")
                            nc.vector.reciprocal_approx_fast(Rf[:], oTs[0:1, :])
                            nc.gpsimd.partition_broadcast(bcs[:], Rf[:])

                        def _norm_tail(p=p, s=s, oTs=oTs, bcs=bcs):
                            tmp = tmp_pool.tile([128, 1024], BF16, tag="tmp",
                                                name="tmp")
                            nc.vector.tensor_mul(
                                tmp[64:128, :], oTs[64:128, :], bcs[64:128, :]
                            )
                            for h in (0, 1):
                                nc.sync.dma_start(
                                    attn_outT[p][64 * h:64 * (h + 1),
                                                 512 * s:512 * (s + 1)],
                                    tmp[64:128, 512 * h:512 * (h + 1)],
                                )
                            if p == 2 and s == NS - 1:
                                # ao[0..2] complete: super-3 partial proj
                                for nt in range(12, 16):
                                    def _pp(nt=nt):
                                        proj_partial(nt)
                                    pending.append(("p", _pp))
                            if p == 3 and s < NS - 1:
                                for nt in range(4 * s, 4 * s + 4):
                                    if s == 2 and nt >= 10:
                                        def _pt(nt=nt):
                                            proj_tail_step(nt, psS, "s2")
                                        tail_fill.append(_pt)
                                    else:
                                        def _pj(nt=nt):
                                            proj_step(nt)
                                        pending.append(("p", _pj))
                        deferred.append((_norm_mid, _norm_tail))
                    else:
                        # last super: psum-direct normalize in 128-token
                        # chunks; reserved proj steps (ScalarE evacuation, no
                        # Vector deps) keep the PE busy through the chain
                        if tail_fill:
                            tail_fill.pop(0)()
                        Rf = rf_pool.tile([1, 1024], F32, tag="rf", name="Rf")
                        nc.vector.reciprocal_approx_fast(Rf[:], oT[0:1, :])
                        bcs = bcs_pool.tile([128, 1024], F32, tag="bcs",
                                            name="bcs")
                        oT3 = oT[:].rearrange("p (h q) -> p h q", h=2)
                        Rf3 = Rf[:].rearrange("o (h q) -> o h q", h=2)
                        bc3 = bcs[:].rearrange("p (h q) -> p h q", h=2)
                        tmp = tmp_pool.tile([128, 1024], BF16, tag="tmp",
                                            name="tmp")
                        t3 = tmp[:].rearrange("p (h q) -> p h q", h=2)
                        for i, nt in enumerate(range(12, 16)):
                            cs = slice(128 * i, 128 * (i + 1))
                            nc.gpsimd.partition_broadcast(
                                bc3[:, :, cs], Rf3[:, :, cs]
                            )
                            nc.vector.tensor_mul(
                                t3[64:128, :, cs], oT3[64:128, :, cs],
                                bc3[64:128, :, cs]
                            )
                            for h in (0, 1):
                                nc.sync.dma_start(
                                    attn_outT[3][64 * h:64 * (h + 1),
                                                 1536 + 128 * i:1664 + 128 * i],
                                    tmp[64:128, 512 * h + 128 * i:
                                        512 * h + 128 * (i + 1)],
                                )
                            if i == 0 and tail_fill:
                                # before the first final: in-order PE would
                                # block it behind F12's DMA wait otherwise
                                tail_fill.pop(0)()
                            proj_final(nt, psS, "s2")
                if p < 3:
                    qT, kT = nq, nk
            for fn in tail_fill:
                fn()
            fill_all()

    nc.compile()
    return nc


def _tril_np():
    import ml_dtypes

    i = np.arange(128)[:, None]
    j = np.arange(128)[None, :]
    return (j >= i).astype(np.float32).astype(ml_dtypes.bfloat16)


def make_in_maps(x, qkv_w, proj_w):
    import ml_dtypes

    bf16 = ml_dtypes.bfloat16
    x = np.asarray(x, dtype=np.float32)
    qkv_w = np.asarray(qkv_w, dtype=np.float32)
    proj_w = np.asarray(proj_w, dtype=np.float32)
    tril = _tril_np()
    in_maps = []
    for c in range(8):
        b, g = c // 2, c % 2
        sl = slice(g * GC, (g + 1) * GC)
        wq, wk, wv = qkv_w[0:C][sl], qkv_w[C:2 * C][sl], qkv_w[2 * C:3 * C][sl]
        in_maps.append(
            {
                "xT": np.ascontiguousarray(x[b].T).astype(bf16),
                "wqkvT": np.ascontiguousarray(
                    np.concatenate([wq, wk, wv], 0).T
                ).astype(bf16),
                "projT": np.ascontiguousarray(proj_w[:, sl].T).astype(bf16),
                "tril": tril,
            }
        )
    return in_maps


def kernel(x, qkv_w, proj_w, proj_b):
    proj_b = np.asarray(proj_b, dtype=np.float32)

    if "nc" not in _cache:
        _cache["nc"] = _build_nc()
    nc = _cache["nc"]

    in_maps = make_in_maps(x, qkv_w, proj_w)
    res = run_bass_kernel_spmd(nc, in_maps, core_ids=list(range(8)))
    out = np.stack(
        [res.results[2 * b]["out"] + res.results[2 * b + 1]["out"] for b in range(B)], 0
    )
    return (out + proj_b[None, None, :]).astype(np.float32)
